# revision 1
# baseline (speedup 1.0000x reference)
"""Corner-detection (structure-tensor min-eigenvalue + edge magnitude)
Bass/Tile kernel for Trainium2, sharded over 8 NeuronCores by image rows.

v4 pipeline per core (512 image rows), per 128-row tile (5 overlapping
tiles, step 97), per half-width stripe (2 stripes of 2048 cols):
  PE : vs = 0.5*vertical-smooth(gray), vi = (3/32)*vertical-interp(gray)
       fused with the grayscale channel mix (banded f16 lhsT per channel,
       PSUM accumulate), chunked N<=512.
  ACT: evacuate vs/vi PSUM -> SBUF f16.
  DVE: ix = vs[c-1]-vs[c+1] (=Ix/2); iy = (10/3)*vi[c] + (vi[c-1]+vi[c+1])
       (=Iy/2, fused scalar_tensor_tensor); pxx=ix^2, pyy=iy^2 (pyy on
       GPSIMD), pxy=ix*iy on GPSIMD; q1=pxx+pyy, q2=pxx-pyy.
       edge = |ix|+|iy| = 0.5(|Ix|+|Iy|) via abs_max tensor_scalar +
       fused scalar_tensor_tensor. No sqrt/abs on ACT for edge.
  PE : TR = 4*box2d(q1) = A+B, DF = 4*box2d(q2) = A-B, C2 = 8*box2d(pxy)
       = 2C via banded lhsT x 3 horizontally-shifted rhs, PSUM accumulate.
  ACT: dd = DF^2, cq = C2^2 (Square straight from PSUM = fused evac);
       ss = sqrt(dd+cq).
  DVE: ee = dd+cq; eig = TR - ss (one PSUM operand, fused evac).

Image-boundary zero-pad semantics of the reference (products are
zero-padded before the box conv) are handled by per-core box-weight
variants (top/bottom row kill) and by zeroed border columns.
"""

import numpy as np

# ---------------------------------------------------------------------------
# BIR patch: this container's walrus build accepts only ONE sync-wait per
# instruction, but Tile's kernel-tail Drain aggregates one wait per logical
# processor.  Split any instruction carrying >1 waits into preceding
# same-engine Drain clones each carrying a single wait.
# ---------------------------------------------------------------------------
import orjson

_MAX_WAITS = 1


def _split_block(insts):
    out = []
    ctr = 0
    for inst in insts:
        si = inst.get("sync_info")
        ow = (si or {}).get("on_wait") or []
        if len(ow) > _MAX_WAITS:
            extra, keep = ow[:-_MAX_WAITS], ow[-_MAX_WAITS:]
            for i in range(0, len(extra), _MAX_WAITS):
                out.append(
                    {
                        "name": f"{inst['name']}-ws{ctr}",
                        "opcode": "Drain",
                        "engine": inst["engine"],
                        "ins": [],
                        "outs": [],
                        "is_reset_sema": False,
                        "debug": inst.get("debug", 0),
                        "sync_info": {
                            "on_update": [],
                            "on_wait": extra[i : i + _MAX_WAITS],
                        },
                    }
                )
                ctr += 1
            si["on_wait"] = keep
        out.append(inst)
    return out


def _split_sem_waits(bir_json: bytes) -> bytes:
    d = orjson.loads(bir_json)
    changed = False
    for fn in d.get("functions", []):
        for blk in fn.get("blocks", []):
            insts = blk.get("instructions", [])
            if any(
                len(((i.get("sync_info") or {}).get("on_wait") or [])) > _MAX_WAITS
                for i in insts
            ):
                blk["instructions"] = _split_block(insts)
                changed = True
    return orjson.dumps(d) if changed else bir_json


def _install_birpatch():
    import concourse.bass_utils as bu
    import concourse.bass2jax as b2j

    if getattr(bu.compile_bir_kernel, "_waitsplit", False):
        return

    orig = bu.compile_bir_kernel

    def patched(bir_json, tmpdir, neff_name="file.neff"):
        return orig(_split_sem_waits(bir_json), tmpdir, neff_name)

    patched._waitsplit = True
    bu.compile_bir_kernel = patched
    b2j.compile_bir_kernel = patched


_install_birpatch()

import concourse.bass as bass
import concourse.tile as tile
from concourse import mybir
from concourse.bass_utils import run_bass_kernel_spmd

# ---------------------------------------------------------------------------
# Geometry constants
# ---------------------------------------------------------------------------
N_CORES = 8
H = W = 4096
BAND = H // N_CORES          # 512 output rows per core
SLAB = 520                   # input rows per core: band + 2 halo top + 6 pad
PW = W + 8                   # host-padded width, image at cols [2, 4098)
NSTRIPE = 2
SW = 2056                    # stripe buffer width (2048 + 4 halo + 4 pad)
SIMG = 2048                  # image cols per stripe
TILE_T = [0, 97, 194, 291, 388]          # tile start row within slab
# per-tile store range in slab-row space [lo, hi)
STORE = [(2, 99), (99, 196), (196, 293), (293, 390), (390, 514)]
GRAD_CHUNKS = [(0, 512), (512, 1024), (1024, 1536), (1536, 2048), (2048, 2052)]
BOX_CHUNKS = [(2, 514), (514, 1026), (1026, 1538), (1538, 2050)]

GRAY_W = np.array([0.2989, 0.587, 0.114], dtype=np.float32)
SMOOTH = np.array([3.0, 10.0, 3.0], dtype=np.float32) / 16.0
INTERP = np.array([1.0, 0.0, -1.0], dtype=np.float32)

F32 = mybir.dt.float32
F16 = mybir.dt.float16
ALU = mybir.AluOpType


def _band_lhsT(coeffs, scale=1.0, m_lo=0, m_hi=128, kill_rows=()):
    """lhsT[k, m] = coeffs[k - m + 1] * scale  for |k-m|<=1, m in [m_lo,m_hi).

    out[m] = sum_k lhsT[k, m] * rhs[k]  (vertical 3-tap conv over partitions).
    kill_rows: k indices to zero entirely (image-boundary pad semantics).
    """
    w = np.zeros((128, 128), dtype=np.float32)
    for m in range(m_lo, m_hi):
        for dk in (-1, 0, 1):
            k = m + dk
            if 0 <= k < 128:
                w[k, m] = coeffs[dk + 1] * scale
    for k in kill_rows:
        w[k, :] = 0.0
    return w


def _weights_for_core(core):
    """All [128,128] f16 lhsT weight matrices for one core."""
    ws = {}
    for c in range(3):
        # grads pre-halved: ix = Ix/2, iy = Iy/2 (edge = |ix|+|iy| directly);
        # the box weights carry the compensating x4 (and x8 for 2C).
        ws[f"wvs{c}"] = _band_lhsT(SMOOTH, scale=0.5 * float(GRAY_W[c]))
        ws[f"wvi{c}"] = _band_lhsT(INTERP, scale=0.5 * float(GRAY_W[c]) * 3.0 / 16.0)
    ones = np.array([1.0, 1.0, 1.0], dtype=np.float32)
    # tile 0 of core 0: slab partition 1 is image row -1 -> exclude from box
    k0 = (1,) if core == 0 else ()
    # tile 4 of core 7: slab partition 126 is image row 4096 -> exclude
    k4 = (126,) if core == N_CORES - 1 else ()
    for sfx, kills in (("t0", k0), ("mid", ()), ("t4", k4)):
        ws[f"wbox4_{sfx}"] = _band_lhsT(ones, 4.0, 2, 126, kills)
        ws[f"wboxm4_{sfx}"] = _band_lhsT(ones, -4.0, 2, 126, kills)
        ws[f"wbox8_{sfx}"] = _band_lhsT(ones, 8.0, 2, 126, kills)
    return {k: v.astype(np.float16) for k, v in ws.items()}


WEIGHT_NAMES = [f"wvs{c}" for c in range(3)] + [f"wvi{c}" for c in range(3)] + [
    f"{b}_{s}" for s in ("t0", "mid", "t4") for b in ("wbox4", "wboxm4", "wbox8")
]


# ---------------------------------------------------------------------------
# Kernel build
# ---------------------------------------------------------------------------
def build_nc(repeats=1, mode="full", qcomb_pe=False, pxx_act=True):
    from contextlib import ExitStack

    nc = bass.Bass("TRN2", target_bir_lowering=False, num_devices=N_CORES)
    xs = nc.declare_dram_parameter("xs", [3, SLAB, PW], F16, isOutput=False)
    wt = {}
    for name in WEIGHT_NAMES:
        wt[name] = nc.declare_dram_parameter(name, [128, 128], F16, isOutput=False)
    # per-(tile,stripe) pieces, full 128 partitions, contiguous destination;
    # host trims the halo rows and reassembles.
    edge_o = nc.declare_dram_parameter(
        "edge", [len(TILE_T), NSTRIPE, 128, SIMG], F16, isOutput=True)
    eig_o = nc.declare_dram_parameter(
        "eig", [len(TILE_T), NSTRIPE, 128, SIMG], F16, isOutput=True)

    import os
    with ExitStack() as ctx:
        tc = ctx.enter_context(
            tile.TileContext(nc, trace_sim=bool(os.environ.get("KERNEL_TRACE_SIM")))
        )
        singles = ctx.enter_context(tc.tile_pool(name="singles", bufs=1))
        xpool = ctx.enter_context(tc.tile_pool(name="x", bufs=2))
        gsb = ctx.enter_context(tc.tile_pool(name="gsb", bufs=2))
        ixy = ctx.enter_context(tc.tile_pool(name="ixy", bufs=2))
        tmp = ctx.enter_context(tc.tile_pool(name="tmp", bufs=2))
        prod = ctx.enter_context(tc.tile_pool(name="prod", bufs=2))
        tailA = ctx.enter_context(tc.tile_pool(name="tailA", bufs=2))
        outp = ctx.enter_context(tc.tile_pool(name="outp", bufs=2))
        psg = ctx.enter_context(tc.tile_pool(name="psg", bufs=2, space="PSUM"))
        psb = ctx.enter_context(tc.tile_pool(name="psb", bufs=1, space="PSUM"))

        cprod = {}
        if mode != "full":
            const_e = singles.tile([128, SW], F16, name="const_e", tag="const_e")
            const_g = singles.tile([128, SW], F16, name="const_g", tag="const_g")
            nc.vector.memset(const_e[:], 0.0)
            nc.vector.memset(const_g[:], 0.0)
        if mode == "nodve":
            for t in ("cq1", "cq2", "cpxy"):
                tt = singles.tile([128, SW], F16, name=t, tag=t)
                nc.vector.memset(tt[:], 0.0)
                cprod[t] = tt

        # weights resident in SBUF
        wsb = {}
        for name in WEIGHT_NAMES:
            t = singles.tile([128, 128], F16, name=name, tag=name)
            nc.sync.dma_start(out=t[:], in_=wt[name][:, :])
            wsb[name] = t

        for _rep in range(repeats):
         for ti, T in enumerate(TILE_T):
             sfx = "t0" if ti == 0 else ("t4" if ti == 4 else "mid")
             box4 = wsb[f"wbox4_{sfx}"]
             boxm4 = wsb[f"wboxm4_{sfx}"]
             box8 = wsb[f"wbox8_{sfx}"]
             for s in range(NSTRIPE):
                 col0 = SIMG * s                      # xs col of stripe buf col 0
                 xt = [
                     xpool.tile([128, SW], F16, tag=f"x{c}", name=f"x{c}")
                     for c in range(3)
                 ]
                 for c in range(3):
                     nc.sync.dma_start(
                         out=xt[c][:],
                         in_=xs[c, T : T + 128, col0 : col0 + SW],
                     )

                 if mode == "dmaonly":
                     nc.sync.dma_start(
                         out=edge_o[ti, s, :, :], in_=const_e[:, 2:2050]
                     )
                     nc.sync.dma_start(
                         out=eig_o[ti, s, :, :], in_=const_g[:, 2:2050]
                     )
                     continue

                 vs_sb = gsb.tile([128, SW], F16, tag="vs")
                 vi_sb = gsb.tile([128, SW], F16, tag="vi")
                 for lo, hi in GRAD_CHUNKS:
                     n = hi - lo
                     vs_ps = psg.tile([128, 512], F32, tag="vs_ps")
                     vi_ps = psg.tile([128, 512], F32, tag="vi_ps")
                     for c in range(3):
                         nc.tensor.matmul(
                             vs_ps[:, :n], wsb[f"wvs{c}"][:], xt[c][:, lo:hi],
                             start=(c == 0), stop=(c == 2),
                         )
                     for c in range(3):
                         nc.tensor.matmul(
                             vi_ps[:, :n], wsb[f"wvi{c}"][:], xt[c][:, lo:hi],
                             start=(c == 0), stop=(c == 2),
                         )
                     nc.scalar.copy(out=vs_sb[:, lo:hi], in_=vs_ps[:, :n])
                     nc.scalar.copy(out=vi_sb[:, lo:hi], in_=vi_ps[:, :n])

                 if mode == "nodve":
                     q1, q2, pxy = cprod["cq1"], cprod["cq2"], cprod["cpxy"]
                     tr_in = [(q1, box4)]
                     df_in = [(q2, box4)]
                     c2_in = [(pxy, box8)]
                 if mode == "full":
                  # shifted convention: tile index j holds the value at
                  # column j+1, so every 16-bit DVE op reads/writes 4B-aligned
                  # step-1 APs and gets its 2x packed mode.
                  ix = ixy.tile([128, SW], F16, tag="ix")
                  iy = ixy.tile([128, SW], F16, tag="iy")
                  t2 = tmp.tile([128, SW], F16, tag="t2")
                  nc.vector.tensor_tensor(
                      ix[:, 0:2050], vs_sb[:, 0:2050], vs_sb[:, 2:2052],
                      ALU.subtract,
                  )
                  nc.vector.tensor_tensor(
                      t2[:, 0:2050], vi_sb[:, 0:2050], vi_sb[:, 2:2052],
                      ALU.add,
                  )
                  nc.vector.scalar_tensor_tensor(
                      out=iy[:, 0:2050], in0=vi_sb[:, 1:2051],
                      scalar=float(10.0 / 3.0), in1=t2[:, 0:2050],
                      op0=ALU.mult, op1=ALU.add,
                  )

                  # products (ix=Ix/2 etc); pxx engine is a balance knob
                  pxx = prod.tile([128, SW], F16, tag="pxx")
                  pyy = prod.tile([128, SW], F16, tag="pyy")
                  pxy = prod.tile([128, SW], F16, tag="pxy")
                  if pxx_act:
                      nc.scalar.square(out=pxx[:, 0:2050], in_=ix[:, 0:2050])
                  else:
                      nc.vector.tensor_tensor(
                          pxx[:, 0:2050], ix[:, 0:2050], ix[:, 0:2050],
                          ALU.mult)
                  nc.vector.tensor_tensor(
                      pyy[:, 0:2050], iy[:, 0:2050], iy[:, 0:2050], ALU.mult)
                  nc.vector.tensor_tensor(
                      pxy[:, 0:2050], ix[:, 0:2050], iy[:, 0:2050], ALU.mult)
                  if qcomb_pe:
                      # TR = box(pxx)+box(pyy), DF = box(pxx)-box(pyy) as
                      # 6-matmul PSUM chains; no q1/q2 DVE passes.
                      tr_in = [(pxx, box4), (pyy, box4)]
                      df_in = [(pxx, box4), (pyy, boxm4)]
                      c2_in = [(pxy, box8)]
                  else:
                      q1 = prod.tile([128, SW], F16, tag="q1")
                      q2 = prod.tile([128, SW], F16, tag="q2")
                      nc.vector.tensor_tensor(
                          q1[:, 0:2050], pxx[:, 0:2050], pyy[:, 0:2050],
                          ALU.add)
                      nc.vector.tensor_tensor(
                          q2[:, 0:2050], pxx[:, 0:2050], pyy[:, 0:2050],
                          ALU.subtract)
                      tr_in = [(q1, box4)]
                      df_in = [(q2, box4)]
                      c2_in = [(pxy, box8)]

                  # edge = |ix| + |iy| = 0.5(|Ix|+|Iy|); |x| = max(-x, x)
                  # as one fused scalar_tensor_tensor per operand.
                  aix = tmp.tile([128, SW], F16, tag="aix")
                  aiy = tmp.tile([128, SW], F16, tag="aiy")
                  nc.vector.scalar_tensor_tensor(
                      out=aix[:, 0:2050], in0=ix[:, 0:2050], scalar=-1.0,
                      in1=ix[:, 0:2050], op0=ALU.mult, op1=ALU.max,
                  )
                  nc.vector.scalar_tensor_tensor(
                      out=aiy[:, 0:2050], in0=iy[:, 0:2050], scalar=-1.0,
                      in1=iy[:, 0:2050], op0=ALU.mult, op1=ALU.max,
                  )
                  edge_sb = outp.tile([128, SW], F16, tag="edge")
                  nc.vector.tensor_tensor(
                      edge_sb[:, 0:2050], aix[:, 0:2050], aiy[:, 0:2050],
                      ALU.add,
                  )
                  nc.sync.dma_start(
                      out=edge_o[ti, s, :, :], in_=edge_sb[:, 1:2049]
                  )

                 # product index j holds image col col0+j-1; outside-image
                 # border columns are excluded by restricting the -1/+1 taps
                 # (the full-width d=0 start tap initializes the whole bank
                 # via has_written, partial taps accumulate) - no memsets.
                 vlo, vhi = (1, 2050) if s == 0 else (0, 2049)

                 def _chain(ps, lo, hi, inputs):
                     taps = []
                     for q, w in inputs:
                         for d in (0, -1, 1):
                             m_lo = max(lo, vlo + 1 - d)
                             m_hi = min(hi, vhi + 1 - d)
                             taps.append((q, w, d, m_lo, m_hi))
                     for i, (q, w, d, m_lo, m_hi) in enumerate(taps):
                         nc.tensor.matmul(
                             ps[:, m_lo - lo : m_hi - lo], w[:],
                             q[:, m_lo - 1 + d : m_hi - 1 + d],
                             start=(i == 0), stop=(i == len(taps) - 1),
                             skip_group_check=True,
                         )

                 dd = tailA.tile([128, SW], F16, tag="dd")
                 cq = tailA.tile([128, SW], F16, tag="cq")
                 ee = tailA.tile([128, SW], F16, tag="ee")
                 ss = tailA.tile([128, SW], F16, tag="ss")
                 eig_sb = outp.tile([128, SW], F16, tag="eig")
                 for lo, hi in BOX_CHUNKS:
                     n = hi - lo
                     tr_ps = psb.tile([128, 512], F32, tag="tr_ps", bufs=2)
                     df_ps = psb.tile([128, 512], F32, tag="df_ps")
                     c2_ps = psb.tile([128, 512], F32, tag="c2_ps")
                     _chain(tr_ps, lo, hi, tr_in)
                     _chain(df_ps, lo, hi, df_in)
                     _chain(c2_ps, lo, hi, c2_in)
                     # fused PSUM evac: dd = DF^2, cq = C2^2 on ACT;
                     # eig = TR - ss with TR read straight from PSUM on DVE
                     nc.scalar.square(out=dd[:, lo:hi], in_=df_ps[:, :n])
                     nc.scalar.square(out=cq[:, lo:hi], in_=c2_ps[:, :n])
                     nc.vector.tensor_tensor(
                         ee[:, lo:hi], dd[:, lo:hi], cq[:, lo:hi], ALU.add)
                     nc.scalar.sqrt(out=ss[:, lo:hi], in_=ee[:, lo:hi])
                     nc.vector.tensor_tensor(
                         eig_sb[:, lo:hi], tr_ps[:, :n], ss[:, lo:hi],
                         ALU.subtract)

                 if mode == "nodve":
                     nc.sync.dma_start(
                         out=edge_o[ti, s, :, :], in_=const_e[:, 2:2050]
                     )
                     nc.sync.dma_start(
                         out=eig_o[ti, s, :, :], in_=const_g[:, 2:2050]
                     )
                     continue

                 nc.sync.dma_start(
                     out=eig_o[ti, s, :, :], in_=eig_sb[:, 2:2050]
                 )
    return nc


_NC_CACHE = None


def _get_nc():
    global _NC_CACHE
    if _NC_CACHE is None:
        _NC_CACHE = build_nc()
    return _NC_CACHE


def kernel(x, edge_filter):
    x = np.asarray(x, dtype=np.float32)
    nc = _get_nc()

    gxp = np.zeros((3, H + 8, PW), dtype=np.float16)
    gxp[:, 2 : 2 + H, 2 : 2 + W] = x[0]

    in_maps = []
    for k in range(N_CORES):
        m = {"xs": np.ascontiguousarray(gxp[:, BAND * k : BAND * k + SLAB, :])}
        m.update(_weights_for_core(k))
        in_maps.append(m)

    res = run_bass_kernel_spmd(nc, in_maps, list(range(N_CORES)))
    edge = np.empty((1, H, W), dtype=np.float32)
    eig = np.empty((1, H, W), dtype=np.float32)
    for k in range(N_CORES):
        for t, T in enumerate(TILE_T):
            lo, hi = STORE[t]
            for s in range(NSTRIPE):
                rows = slice(BAND * k + lo - 2, BAND * k + hi - 2)
                cols = slice(SIMG * s, SIMG * (s + 1))
                edge[0, rows, cols] = res.results[k]["edge"][
                    t, s, lo - T : hi - T].astype(np.float32)
                eig[0, rows, cols] = res.results[k]["eig"][
                    t, s, lo - T : hi - T].astype(np.float32)
    return (edge, eig)

